# revision 10
# baseline (speedup 1.0000x reference)
"""
DistancePredictor Trainium2 kernel.

Math:
  xi = x @ Wi + bi            [B, L, H]
  xj = x @ Wj + bj            [B, L, H]
  out = relu(xi[:,:,None,:] * xj[:,None,:,:]) @ Wo + bo    [B, L, L, NB]

Key identity (exact, terms have disjoint support):
  relu(a*b) = relu(a)relu(b) + relu(-a)relu(-b)
so
  out[i,j,n] = sum_h (A+[i,h]B+[j,h] + A-[i,h]B-[j,h]) * Wo[h,n] + bo[n]
with A± = relu(±xi), B± = relu(±xj) — the whole pair/relu/contract
pipeline is pure TensorE matmuls; no [B,L,L,H] intermediate exists.

Sharding: 8 cores; core c handles batch b=c//4 and i-rows
[96*(c%4), 96*(c%4)+96).  Weights replicated.

Schedule (v2 — k-major unified stream):
 - All inputs stream as 10 chunk triggers chk[k] = [wi_k | wj_k | x_k]
   (229KB each, 1792B/partition descriptors), alternating the two HWDGE
   rings.  Chunk k's arrival enables ALL of layer-1 for contraction
   chunk k (psA t0/t1 + psB t0/t1, single-pass LDWEIGHTS), so layer 1
   is stream-paced and finishes ~0.5us after the last byte.
 - cst/bias rows ride the gpsimd SWDGE path (off the HW rings).
 - Biases enter the PSUM accumulations as rank-1 matmuls (ones ⊗ b).
 - Junk matmuls on a memset tile ramp the HAM clock before chunk 0.
 - at±-chain split: Vector owns t0 (fused PSUM reads), GpSimd owns t1
   (from SBUF relu copies), Scalar owns am1/bp relus.
 - Main contraction is j-block major: stationary = b±t j-block (M=128),
   moving = at± [n-half, i] (N=480).
 - bo + fp32->fp16 conversion fuse into one Vector broadcast-add per
   output block; blocks drain on alternating rings during the main loop.
"""

import numpy as np

import concourse.bass as bass
import concourse.mybir as mybir
import concourse.tile as tile
from concourse import bacc, bass_utils

# Problem constants (hardcoded per contract).
B, L, D, H, NB = 2, 384, 1280, 256, 10
P = 128
KT = D // P     # 10 contraction chunks of 128
HT = H // P     # 2 h-chunks of 128
NCORES = 8
IB = (B * L) // NCORES   # 96 i-rows per core
CW = 2 * H + L           # chunk width: wi(256) | wj(256) | x(384)

F32 = mybir.dt.float32
F16 = mybir.dt.float16
ALU = mybir.AluOpType
RELU = mybir.ActivationFunctionType.Relu

_last_result = None  # BassKernelResults of the most recent run (for test harness)


def build_nc():
    nc = bacc.Bacc("TRN2")

    # chunk k: per partition [wi_k(0:256) | wj_k(256:512) | x_k(512:896)]
    chk = nc.dram_tensor("chk", [KT, P, CW], F16, kind="ExternalInput")
    # cst[:, 0:2] = Wo per h-chunk, [:, 2:4] = -Wo, [:, 4] = bo replicated
    cst = nc.dram_tensor("cst", [P, 5, NB], F32, kind="ExternalInput")
    # bias rows on one partition: [bi_t0, bi_t1, bj_t0, bj_t1]
    brow = nc.dram_tensor("brow", [1, 4, P], F16, kind="ExternalInput")
    # [n-half, j-block, j, n, i]: output in j-major blocks (M=128 matmuls)
    out = nc.dram_tensor("out", [2, 3, P, NB // 2, IB], F16, kind="ExternalOutput")

    with tile.TileContext(nc) as tc:
        with (
            tc.tile_pool(name="persist", bufs=1) as pp,
            tc.tile_pool(name="psA", bufs=2, space="PSUM") as psA_pool,
            tc.tile_pool(name="psB", bufs=2, space="PSUM") as psB_pool,
            tc.tile_pool(name="psO", bufs=4, space="PSUM") as psO_pool,
            tc.tile_pool(name="stage", bufs=6) as stage_pool,
        ):
            tl = lambda shape, name, dt=F32: pp.tile(shape, dt, name=name, tag=name)
            chk_sb = tl([P, KT, CW], "chk_sb", F16)
            cst_sb = tl([P, 5, NB], "cst_sb")
            brow_sb = tl([1, 4, P], "brow_sb", F16)
            ones_sb = tl([1, L], "ones_sb", F16)

            bp_sb = tl([P, HT, L], "bp_sb", F16)         # relu(xj+bj)      [h, j]
            bm_sb = tl([P, HT, L], "bm_sb", F16)         # relu(-(xj+bj))
            atp_sb = tl([P, HT, NB, IB], "atp_sb", F16)  # [h, n, i]
            atm_sb = tl([P, HT, NB, IB], "atm_sb", F16)
            ap1_sb = tl([P, IB], "ap1_sb")               # max(psA1, 0)
            am1_sb = tl([P, IB], "am1_sb")               # relu(-psA1)

            warm_sb = tl([P, L], "warm_sb", F16)
            # memsets on gpsimd: it is idle at program start, so the first
            # junk matmul (and the HAM clock ramp) starts ~1.5us earlier
            # than with vector memsets (vector waits on engine-main entry).
            nc.gpsimd.memset(warm_sb[:], 0.0)
            nc.gpsimd.memset(ones_sb[:], 1.0)

            def junk(n_junk):
                # Full 128-partition matmuls: the HAM clock monitor only
                # counts wide-K PE streaming (K=32/64 never ramps).
                psW = psO_pool.tile([IB, L], F32, name="psW", tag="psO")
                for _ in range(n_junk):
                    nc.tensor.matmul(psW[:], warm_sb[:, :IB], warm_sb[:],
                                     start=True, stop=True,
                                     skip_group_check=True)

            # ---- DMA triggers.  Emission order per engine = issue order.
            # Chunk triggers alternate the two HWDGE rings, FIFO per ring,
            # so data arrives roughly pairwise in k order.  brow rides
            # early on the scalar ring (bias matmuls run mid-stream);
            # cst rides late on the sync ring (needed only by the
            # post-op chain).
            nc.sync.dma_start(chk_sb[:, 0, :], chk[0])
            nc.scalar.dma_start(chk_sb[:, 1, :], chk[1])
            nc.scalar.dma_start(brow_sb[:], brow[:])
            for k in range(2, KT):
                eng = nc.sync if k % 2 == 0 else nc.scalar
                if k == 8:
                    nc.sync.dma_start(cst_sb[:], cst[:])
                eng.dma_start(chk_sb[:, k, :], chk[k])

            psA = [psA_pool.tile([P, IB], F32, name="psA", tag="psA")
                   for _ in range(HT)]
            psB = [psB_pool.tile([P, L], F32, name="psB", tag="psB")
                   for _ in range(HT)]

            junk(6)

            # ---- layer 1: chunk-paced, consume chunks in arrival order.
            # k=0 opens the PSUM accumulation groups; bias rank-1 matmuls
            # join after k=2 (once brow has landed); on the last chunk
            # psA closes first so the at±-chain overlaps the psB tail.
            def a_mm(k, t, start=False, stop=False):
                nc.tensor.matmul(psA[t][:], chk_sb[:, k, t * P:(t + 1) * P],
                                 chk_sb[:, k, 2 * H:2 * H + IB],
                                 start=start, stop=stop)

            def b_mm(k, t, start=False, stop=False):
                nc.tensor.matmul(psB[t][:],
                                 chk_sb[:, k, H + t * P:H + (t + 1) * P],
                                 chk_sb[:, k, 2 * H:],
                                 start=start, stop=stop)

            for k in range(KT):
                if k < KT - 1:
                    for t in range(HT):
                        a_mm(k, t, start=k == 0)
                    for t in range(HT):
                        b_mm(k, t, start=k == 0)
                else:
                    a_mm(k, 0, stop=True)
                    a_mm(k, 1, stop=True)
                    b_mm(k, 0, stop=True)
                    b_mm(k, 1, stop=True)
                if k == 2:
                    # bias rank-1 joins each accumulation group
                    for t in range(HT):
                        nc.tensor.matmul(psA[t][:], brow_sb[:, t],
                                         ones_sb[:, :IB],
                                         start=False, stop=False)
                    for t in range(HT):
                        nc.tensor.matmul(psB[t][:], brow_sb[:, 2 + t],
                                         ones_sb[:],
                                         start=False, stop=False)
                if 2 <= k < KT - 1:
                    # once the clock has ramped the PE outruns the stream
                    # (~450ns/chunk vs ~700ns arrival); junk-fill the gap
                    # so the HAM clock governor never sags mid-stream
                    junk(1)

            # ---- fused post-ops ----
            wo_b = lambda s, lo, hi: cst_sb[:, s, lo:hi, None].to_broadcast(
                (P, hi - lo, IB))
            psa_b = lambda lo, hi: psA[0][:, None, :].to_broadcast(
                (P, hi - lo, IB))

            def at0_op(sign, lo, hi):
                # fused t0: atp = max(psA,0)*Wo ; atm = min(psA,0)*(-Wo)
                dst = (atp_sb if sign == 0 else atm_sb)[:, 0, lo:hi]
                op0 = ALU.max if sign == 0 else ALU.min
                nc.vector.scalar_tensor_tensor(dst, psa_b(lo, hi), 0.0,
                                               wo_b(2 * sign, lo, hi),
                                               op0, ALU.mult)

            def at1_op(sign, lo, hi):
                # t1 on gpsimd from SBUF; both a-parts non-negative -> +Wo.
                src = ap1_sb if sign == 0 else am1_sb
                dst = (atp_sb if sign == 0 else atm_sb)[:, 1, lo:hi]
                nc.gpsimd.tensor_tensor(
                    dst, src[:, None, :].to_broadcast((P, hi - lo, IB)),
                    wo_b(1, lo, hi), ALU.mult)

            # Emission order = scheduler priority: the scalar trio goes
            # FIRST so its waits chain to the PE stop semaphores directly
            # instead of being serialized behind the vector queue.
            # scalar: am1 copy + bp relus
            nc.scalar.activation(am1_sb[:], psA[1][:], RELU, scale=-1.0)
            nc.scalar.activation(bp_sb[:, 0], psB[0][:], RELU)
            nc.scalar.activation(bp_sb[:, 1], psB[1][:], RELU)
            # vector: t0 chain + ap1 + bm relus, ordered so the main
            # loop's first consumers (nh0) are produced first
            at0_op(0, 0, 5)   # + side first: the main loop consumes it first
            nc.vector.tensor_scalar_max(ap1_sb[:], psA[1][:], 0.0)
            at0_op(1, 0, 5)
            nc.vector.tensor_scalar(bm_sb[:, 0], psB[0][:], -1.0, 0.0,
                                    ALU.mult, ALU.max)
            nc.vector.tensor_scalar(bm_sb[:, 1], psB[1][:], -1.0, 0.0,
                                    ALU.mult, ALU.max)
            at0_op(0, 5, 10)
            at0_op(1, 5, 10)
            # gpsimd: t1 chain
            at1_op(0, 0, 5)
            at1_op(1, 0, 5)
            at1_op(0, 5, 10)
            at1_op(1, 5, 10)

            # ---- main contraction, j-block major: stationary = b±t j-block
            # (M=128), moving = at± [n-half, i] (N=480).  Output bias
            # enters via cst row 4 during the fp32->fp16 PSUM drain.
            NH2 = NB // 2
            atp_v = atp_sb[:].rearrange("p t n i -> p t (n i)")
            atm_v = atm_sb[:].rearrange("p t n i -> p t (n i)")
            junk(1)
            for idx in range(6):
                nh, jb = idx // 3, idx % 3
                ns = slice(nh * NH2 * IB, (nh + 1) * NH2 * IB)
                js = slice(jb * P, (jb + 1) * P)
                psO = psO_pool.tile([P, NH2 * IB], F32, name="psO", tag="psO")
                nc.tensor.matmul(psO[:], bp_sb[:, 0, js], atp_v[:, 0, ns],
                                 start=True, stop=False)
                if idx == 0:
                    junk(2)  # bridge the at-chain latency, keep the clock up
                nc.tensor.matmul(psO[:], bm_sb[:, 0, js], atm_v[:, 0, ns],
                                 start=False, stop=False)
                if idx == 0:
                    junk(1)
                nc.tensor.matmul(psO[:], bp_sb[:, 1, js], atp_v[:, 1, ns],
                                 start=False, stop=False)
                if idx == 0:
                    junk(1)
                nc.tensor.matmul(psO[:], bm_sb[:, 1, js], atm_v[:, 1, ns],
                                 start=False, stop=True)
                # bias rides the fp16 convert as a broadcast add on Vector
                # (cst row 4 = bo replicated across partitions); no PE time.
                ostage = stage_pool.tile([P, NH2, IB], F16, name="ostage",
                                         tag="ostage")
                psO_3 = psO[:].rearrange("p (n i) -> p n i", i=IB)
                bo_b = lambda lo, hi: cst_sb[:, 4, nh * NH2 + lo:nh * NH2 + hi,
                                             None].to_broadcast((P, hi - lo, IB))
                eng = nc.scalar if idx % 2 == 0 else nc.sync
                if idx == 5:
                    # last block: convert/DMA in halves so the first half
                    # streams out while the second converts
                    nc.vector.tensor_tensor(ostage[:, 0:2], psO_3[:, 0:2],
                                            bo_b(0, 2), ALU.add)
                    eng.dma_start(out[nh, jb, :, 0:2], ostage[:, 0:2])
                    nc.vector.tensor_tensor(ostage[:, 2:NH2], psO_3[:, 2:NH2],
                                            bo_b(2, NH2), ALU.add)
                    nc.scalar.dma_start(out[nh, jb, :, 2:NH2], ostage[:, 2:NH2])
                else:
                    nc.vector.tensor_tensor(ostage[:], psO_3, bo_b(0, NH2),
                                            ALU.add)
                    eng.dma_start(out[nh, jb], ostage[:])

    return nc


def _prep_inputs(x, Wi, bi, Wj, bj, Wo, bo):
    """Build the 8 per-core input maps."""
    f = lambda a: np.ascontiguousarray(np.asarray(a, dtype=np.float32))
    x, Wi, bi, Wj, bj, Wo, bo = map(f, (x, Wi, bi, Wj, bj, Wo, bo))

    wi_k = Wi.astype(np.float16).reshape(KT, P, H)      # [k, 128, 256]
    wj_k = Wj.astype(np.float16).reshape(KT, P, H)

    wo_r = Wo.reshape(HT, P, NB).transpose(1, 0, 2)            # [128, 2, 10]
    cst = np.ascontiguousarray(np.stack(
        [wo_r[:, 0], wo_r[:, 1], -wo_r[:, 0], -wo_r[:, 1],
         np.tile(bo[None, :], (P, 1))], axis=1)).astype(np.float32)  # [128, 5, 10]
    brow = np.concatenate([bi.reshape(HT, P), bj.reshape(HT, P)],
                          axis=0)[None].astype(np.float16)     # [1, 4, 128]
    brow = np.ascontiguousarray(brow)

    xT = [x[b].T for b in range(B)]                            # [1280, 384]
    in_maps = []
    for c in range(NCORES):
        b, i0 = c // (NCORES // B), (c % (NCORES // B)) * IB
        xc = np.roll(xT[b], -i0, axis=1).astype(np.float16)    # i-cols first
        xk = xc.reshape(KT, P, L)                              # [k, 128, 384]
        chk = np.concatenate([wi_k, wj_k, xk], axis=2)         # [k, 128, 896]
        chk = np.ascontiguousarray(chk)
        in_maps.append({"chk": chk, "cst": cst, "brow": brow})
    return in_maps


def _run(inputs, trace=False):
    global _last_result
    nc = build_nc()
    if not nc.is_finalized():
        nc.finalize()
    in_maps = _prep_inputs(**inputs)
    res = bass_utils.run_bass_kernel_spmd(
        nc, in_maps, core_ids=list(range(NCORES)), trace=trace)
    _last_result = res
    full = np.empty((B, L, L, NB), dtype=np.float32)
    for c in range(NCORES):
        b, i0 = c // (NCORES // B), (c % (NCORES // B)) * IB
        o = res.results[c]["out"].astype(np.float32)   # [2, 3, 128, 5, 96]
        o = o.transpose(4, 1, 2, 0, 3).reshape(IB, L, NB)  # -> [i, j_rolled, n]
        full[b, i0:i0 + IB] = np.roll(o, i0, axis=1)
    return full


def kernel(**inputs):
    return _run(inputs, trace=False)


# revision 17
# speedup vs baseline: 1.0325x; 1.0325x over previous
"""
DistancePredictor Trainium2 kernel.

Math:
  xi = x @ Wi + bi            [B, L, H]
  xj = x @ Wj + bj            [B, L, H]
  out = relu(xi[:,:,None,:] * xj[:,None,:,:]) @ Wo + bo    [B, L, L, NB]

Key identity (exact, terms have disjoint support):
  relu(a*b) = relu(a)relu(b) + relu(-a)relu(-b)
so
  out[i,j,n] = sum_h (A+[i,h]B+[j,h] + A-[i,h]B-[j,h]) * Wo[h,n] + bo[n]
with A± = relu(±xi), B± = relu(±xj) — the whole pair/relu/contract
pipeline is pure TensorE matmuls; no [B,L,L,H] intermediate exists.

Sharding: 8 cores; core c handles batch b=c//4 and i-rows
[96*(c%4), 96*(c%4)+96).  Weights replicated.

Schedule (v2 — k-major unified stream):
 - All inputs stream as 10 chunk triggers chk[k] = [wi_k | wj_k | x_k]
   (229KB each, 1792B/partition descriptors), alternating the two HWDGE
   rings.  Chunk k's arrival enables ALL of layer-1 for contraction
   chunk k (psA t0/t1 + psB t0/t1, single-pass LDWEIGHTS), so layer 1
   is stream-paced and finishes ~0.5us after the last byte.
 - cst/bias rows ride the gpsimd SWDGE path (off the HW rings).
 - Biases enter the PSUM accumulations as rank-1 matmuls (ones ⊗ b).
 - Junk matmuls on a memset tile ramp the HAM clock before chunk 0.
 - at±-chain split: Vector owns t0 (fused PSUM reads), GpSimd owns t1
   (from SBUF relu copies), Scalar owns am1/bp relus.
 - Main contraction is j-block major: stationary = b±t j-block (M=128),
   moving = at± [n-half, i] (N=480).
 - bo + fp32->fp16 conversion fuse into one Vector broadcast-add per
   output block; blocks drain on alternating rings during the main loop.
"""

import numpy as np

import concourse.bass as bass
import concourse.mybir as mybir
import concourse.tile as tile
from concourse import bacc, bass_utils

# Problem constants (hardcoded per contract).
B, L, D, H, NB = 2, 384, 1280, 256, 10
P = 128
KT = D // P     # 10 contraction chunks of 128
HT = H // P     # 2 h-chunks of 128
NCORES = 8
IB = (B * L) // NCORES   # 96 i-rows per core
CW = 2 * H + L           # chunk width: wi(256) | wj(256) | x(384)

F32 = mybir.dt.float32
F16 = mybir.dt.float16
ALU = mybir.AluOpType
RELU = mybir.ActivationFunctionType.Relu

_last_result = None  # BassKernelResults of the most recent run (for test harness)


def build_nc():
    nc = bacc.Bacc("TRN2")

    # A-chunk pair g (k=2g,2g+1): per partition [wi_k(0:256) | xi_k(256:352)]
    cha = nc.dram_tensor("cha", [KT // 2, P, 2, 352], F16, kind="ExternalInput")
    # B-chunk pair g: per partition [wj_k(0:256) | x_k(256:640)]
    chb = nc.dram_tensor("chb", [KT // 2, P, 2, 640], F16, kind="ExternalInput")
    # cst[:, 0:2] = Wo per h-chunk, [:, 2:4] = -Wo, [:, 4] = bo replicated
    cst = nc.dram_tensor("cst", [P, 5, NB], F32, kind="ExternalInput")
    # bias rows on one partition: [bi_t0, bi_t1, bj_t0, bj_t1]
    brow = nc.dram_tensor("brow", [1, 4, P], F16, kind="ExternalInput")
    # [n-half, j-block, j, n, i]: output in j-major blocks (M=128 matmuls)
    out = nc.dram_tensor("out", [2, 3, P, NB // 2, IB], F16, kind="ExternalOutput")

    with tile.TileContext(nc) as tc:
        with (
            tc.tile_pool(name="persist", bufs=1) as pp,
            tc.tile_pool(name="psA", bufs=2, space="PSUM") as psA_pool,
            tc.tile_pool(name="psB", bufs=2, space="PSUM") as psB_pool,
            tc.tile_pool(name="psO", bufs=4, space="PSUM") as psO_pool,
            tc.tile_pool(name="stage", bufs=6) as stage_pool,
        ):
            tl = lambda shape, name, dt=F32: pp.tile(shape, dt, name=name, tag=name)
            cha_sb = tl([P, KT // 2, 2, 352], "cha_sb", F16)
            chb_sb = tl([P, KT // 2, 2, 640], "chb_sb", F16)
            cst_sb = tl([P, 5, NB], "cst_sb")
            brow_sb = tl([1, 4, P], "brow_sb", F16)
            ones_sb = tl([1, L], "ones_sb", F16)

            bp_sb = tl([P, HT, L], "bp_sb", F16)         # relu(xj+bj)      [h, j]
            bm_sb = tl([P, HT, L], "bm_sb", F16)         # relu(-(xj+bj))
            atp_sb = tl([P, HT, NB, IB], "atp_sb", F16)  # [h, n, i]
            atm_sb = tl([P, HT, NB, IB], "atm_sb", F16)
            ap1_sb = tl([P, IB], "ap1_sb")               # max(psA1, 0)
            am1_sb = tl([P, IB], "am1_sb")               # relu(-psA1)

            warm_sb = tl([P, L], "warm_sb", F16)
            # memsets on gpsimd: it is idle at program start, so the first
            # junk matmul (and the HAM clock ramp) starts ~1.5us earlier
            # than with vector memsets (vector waits on engine-main entry).
            nc.gpsimd.memset(warm_sb[:], 0.0)
            nc.gpsimd.memset(ones_sb[:], 1.0)

            def junk(n_junk):
                # Full 128-partition matmuls: the HAM clock monitor only
                # counts wide-K PE streaming (K=32/64 never ramps).
                psW = psO_pool.tile([IB, L], F32, name="psW", tag="psO")
                for _ in range(n_junk):
                    nc.tensor.matmul(psW[:], warm_sb[:, :IB], warm_sb[:],
                                     start=True, stop=True,
                                     skip_group_check=True)

            # ---- DMA triggers.  Emission order per engine = issue order.
            # Two-phase stream: A-chunks (wi + i-cols of x) first so psA
            # closes ~3us in and the long at±-chain overlaps the B-phase;
            # B-chunks (wj + full x) stream-pace psB, which closes right
            # before the main loop.  brow lands early on the scalar ring
            # (bias matmuls run mid-A-phase); cst mid-A on sync (needed
            # by the at±-chain at ~A-end).
            nc.sync.dma_start(cha_sb[:, 0], cha[0])
            nc.scalar.dma_start(cha_sb[:, 1], cha[1])
            nc.scalar.dma_start(brow_sb[:], brow[:])
            nc.sync.dma_start(cha_sb[:, 2], cha[2])
            nc.sync.dma_start(cst_sb[:], cst[:])
            nc.scalar.dma_start(cha_sb[:, 3], cha[3])
            nc.sync.dma_start(cha_sb[:, 4], cha[4])
            for g in range(KT // 2):
                eng = nc.scalar if g % 2 == 0 else nc.sync
                eng.dma_start(chb_sb[:, g], chb[g])

            psA = [psA_pool.tile([P, IB], F32, name="psA", tag="psA")
                   for _ in range(HT)]
            psB = [psB_pool.tile([P, L], F32, name="psB", tag="psB")
                   for _ in range(HT)]

            junk(6)

            # ---- layer 1 A-side: psA chunk-paced on the A-stream; bias
            # rank-1 joins after group 0; psA[0] closes first so the
            # Vector t0-chain starts while the t1 matmuls finish.
            def a_mm(g, j, t, start=False, stop=False):
                nc.tensor.matmul(psA[t][:], cha_sb[:, g, j, t * P:(t + 1) * P],
                                 cha_sb[:, g, j, H:H + IB],
                                 start=start, stop=stop)

            def b_mm(g, j, t, start=False, stop=False):
                nc.tensor.matmul(psB[t][:],
                                 chb_sb[:, g, j, t * P:(t + 1) * P],
                                 chb_sb[:, g, j, H:],
                                 start=start, stop=stop)

            for g in range(KT // 2):
                last = g == KT // 2 - 1
                if not last:
                    for j in range(2):
                        for t in range(HT):
                            a_mm(g, j, t, start=g == 0 and j == 0)
                else:
                    # close t0 before t1 so the t0 post-ops start first
                    a_mm(g, 0, 0)
                    a_mm(g, 0, 1)
                    a_mm(g, 1, 0, stop=True)
                    a_mm(g, 1, 1, stop=True)
                if g == 0:
                    for t in range(HT):
                        nc.tensor.matmul(psA[t][:], brow_sb[:, t],
                                         ones_sb[:, :IB],
                                         start=False, stop=False)
                junk(1)

            # ---- at±-chain: emitted here (= high scheduler priority) so
            # it runs concurrently with the B-phase stream/matmuls.
            wo_b = lambda s, lo, hi: cst_sb[:, s, lo:hi, None].to_broadcast(
                (P, hi - lo, IB))
            psa_b = lambda lo, hi: psA[0][:, None, :].to_broadcast(
                (P, hi - lo, IB))

            def at0_op(sign, lo, hi):
                # fused t0: atp = max(psA,0)*Wo ; atm = min(psA,0)*(-Wo)
                dst = (atp_sb if sign == 0 else atm_sb)[:, 0, lo:hi]
                op0 = ALU.max if sign == 0 else ALU.min
                nc.vector.scalar_tensor_tensor(dst, psa_b(lo, hi), 0.0,
                                               wo_b(2 * sign, lo, hi),
                                               op0, ALU.mult)

            def at1_op(sign, lo, hi):
                # t1 on gpsimd from SBUF; both a-parts non-negative -> +Wo.
                src = ap1_sb if sign == 0 else am1_sb
                dst = (atp_sb if sign == 0 else atm_sb)[:, 1, lo:hi]
                nc.gpsimd.tensor_tensor(
                    dst, src[:, None, :].to_broadcast((P, hi - lo, IB)),
                    wo_b(1, lo, hi), ALU.mult)

            nc.scalar.activation(am1_sb[:], psA[1][:], RELU, scale=-1.0)
            at0_op(0, 0, 5)
            nc.vector.tensor_scalar_max(ap1_sb[:], psA[1][:], 0.0)
            at0_op(1, 0, 5)
            at0_op(0, 5, 10)
            at0_op(1, 5, 10)
            at1_op(0, 0, 5)
            at1_op(1, 0, 5)
            at1_op(0, 5, 10)
            at1_op(1, 5, 10)

            # ---- layer 1 B-side: psB chunk-paced on the B-stream ----
            for g in range(KT // 2):
                last = g == KT // 2 - 1
                if not last:
                    for j in range(2):
                        for t in range(HT):
                            b_mm(g, j, t, start=g == 0 and j == 0)
                else:
                    b_mm(g, 0, 0)
                    b_mm(g, 0, 1)
                    b_mm(g, 1, 0, stop=True)
                    b_mm(g, 1, 1, stop=True)
                if g == 0:
                    for t in range(HT):
                        nc.tensor.matmul(psB[t][:], brow_sb[:, 2 + t],
                                         ones_sb[:],
                                         start=False, stop=False)
                junk(2)

            # ---- b± relus: scalar owns bp, vector owns bm ----
            nc.scalar.activation(bp_sb[:, 0], psB[0][:], RELU)
            nc.scalar.activation(bp_sb[:, 1], psB[1][:], RELU)
            nc.vector.tensor_scalar(bm_sb[:, 0], psB[0][:], -1.0, 0.0,
                                    ALU.mult, ALU.max)
            nc.vector.tensor_scalar(bm_sb[:, 1], psB[1][:], -1.0, 0.0,
                                    ALU.mult, ALU.max)

            # ---- main contraction, j-block major: stationary = b±t j-block
            # (M=128), moving = at± [n-half, i] (N=480).  Output bias
            # enters via cst row 4 during the fp32->fp16 PSUM drain.
            NH2 = NB // 2
            atp_v = atp_sb[:].rearrange("p t n i -> p t (n i)")
            atm_v = atm_sb[:].rearrange("p t n i -> p t (n i)")
            junk(1)
            for idx in range(6):
                nh, jb = idx // 3, idx % 3
                ns = slice(nh * NH2 * IB, (nh + 1) * NH2 * IB)
                js = slice(jb * P, (jb + 1) * P)
                psO = psO_pool.tile([P, NH2 * IB], F32, name="psO", tag="psO")
                nc.tensor.matmul(psO[:], bp_sb[:, 0, js], atp_v[:, 0, ns],
                                 start=True, stop=False)
                if idx == 0:
                    junk(2)  # bridge the at-chain latency, keep the clock up
                nc.tensor.matmul(psO[:], bm_sb[:, 0, js], atm_v[:, 0, ns],
                                 start=False, stop=False)
                if idx == 0:
                    junk(1)
                nc.tensor.matmul(psO[:], bp_sb[:, 1, js], atp_v[:, 1, ns],
                                 start=False, stop=False)
                if idx == 0:
                    junk(1)
                nc.tensor.matmul(psO[:], bm_sb[:, 1, js], atm_v[:, 1, ns],
                                 start=False, stop=True)
                # bias rides the fp16 convert as a broadcast add on Vector
                # (cst row 4 = bo replicated across partitions); no PE time.
                ostage = stage_pool.tile([P, NH2, IB], F16, name="ostage",
                                         tag="ostage")
                psO_3 = psO[:].rearrange("p (n i) -> p n i", i=IB)
                bo_b = lambda lo, hi: cst_sb[:, 4, nh * NH2 + lo:nh * NH2 + hi,
                                             None].to_broadcast((P, hi - lo, IB))
                eng = nc.scalar if idx % 2 == 0 else nc.sync
                if idx == 5:
                    # last block: convert/DMA in halves so the first half
                    # streams out while the second converts
                    nc.vector.tensor_tensor(ostage[:, 0:2], psO_3[:, 0:2],
                                            bo_b(0, 2), ALU.add)
                    eng.dma_start(out[nh, jb, :, 0:2], ostage[:, 0:2])
                    nc.vector.tensor_tensor(ostage[:, 2:NH2], psO_3[:, 2:NH2],
                                            bo_b(2, NH2), ALU.add)
                    nc.scalar.dma_start(out[nh, jb, :, 2:NH2], ostage[:, 2:NH2])
                else:
                    nc.vector.tensor_tensor(ostage[:], psO_3, bo_b(0, NH2),
                                            ALU.add)
                    eng.dma_start(out[nh, jb], ostage[:])

    return nc


def _prep_inputs(x, Wi, bi, Wj, bj, Wo, bo):
    """Build the 8 per-core input maps."""
    f = lambda a: np.ascontiguousarray(np.asarray(a, dtype=np.float32))
    x, Wi, bi, Wj, bj, Wo, bo = map(f, (x, Wi, bi, Wj, bj, Wo, bo))

    wi_k = Wi.astype(np.float16).reshape(KT, P, H)      # [k, 128, 256]
    wj_k = Wj.astype(np.float16).reshape(KT, P, H)

    wo_r = Wo.reshape(HT, P, NB).transpose(1, 0, 2)            # [128, 2, 10]
    cst = np.ascontiguousarray(np.stack(
        [wo_r[:, 0], wo_r[:, 1], -wo_r[:, 0], -wo_r[:, 1],
         np.tile(bo[None, :], (P, 1))], axis=1)).astype(np.float32)  # [128, 5, 10]
    brow = np.concatenate([bi.reshape(HT, P), bj.reshape(HT, P)],
                          axis=0)[None].astype(np.float16)     # [1, 4, 128]
    brow = np.ascontiguousarray(brow)

    xT = [x[b].T for b in range(B)]                            # [1280, 384]
    in_maps = []
    for c in range(NCORES):
        b, i0 = c // (NCORES // B), (c % (NCORES // B)) * IB
        xc = np.roll(xT[b], -i0, axis=1).astype(np.float16)    # i-cols first
        xk = xc.reshape(KT, P, L)                              # [k, 128, 384]
        # A-pairs: [wi_k | xi_k] ; B-pairs: [wj_k | x_k]
        cha = np.concatenate([wi_k, xk[:, :, :IB]], axis=2)    # [k, 128, 352]
        cha = np.ascontiguousarray(
            cha.reshape(KT // 2, 2, P, 352).transpose(0, 2, 1, 3))
        chb = np.concatenate([wj_k, xk], axis=2)               # [k, 128, 640]
        chb = np.ascontiguousarray(
            chb.reshape(KT // 2, 2, P, 640).transpose(0, 2, 1, 3))
        in_maps.append({"cha": cha, "chb": chb, "cst": cst, "brow": brow})
    return in_maps


def _run(inputs, trace=False):
    global _last_result
    nc = build_nc()
    if not nc.is_finalized():
        nc.finalize()
    in_maps = _prep_inputs(**inputs)
    res = bass_utils.run_bass_kernel_spmd(
        nc, in_maps, core_ids=list(range(NCORES)), trace=trace)
    _last_result = res
    full = np.empty((B, L, L, NB), dtype=np.float32)
    for c in range(NCORES):
        b, i0 = c // (NCORES // B), (c % (NCORES // B)) * IB
        o = res.results[c]["out"].astype(np.float32)   # [2, 3, 128, 5, 96]
        o = o.transpose(4, 1, 2, 0, 3).reshape(IB, L, NB)  # -> [i, j_rolled, n]
        full[b, i0:i0 + IB] = np.roll(o, i0, axis=1)
    return full


def kernel(**inputs):
    return _run(inputs, trace=False)


# revision 18
# speedup vs baseline: 1.0894x; 1.0552x over previous
"""
DistancePredictor Trainium2 kernel.

Math:
  xi = x @ Wi + bi            [B, L, H]
  xj = x @ Wj + bj            [B, L, H]
  out = relu(xi[:,:,None,:] * xj[:,None,:,:]) @ Wo + bo    [B, L, L, NB]

Key identity (exact, terms have disjoint support):
  relu(a*b) = relu(a)relu(b) + relu(-a)relu(-b)
so
  out[i,j,n] = sum_h (A+[i,h]B+[j,h] + A-[i,h]B-[j,h]) * Wo[h,n] + bo[n]
with A± = relu(±xi), B± = relu(±xj) — the whole pair/relu/contract
pipeline is pure TensorE matmuls; no [B,L,L,H] intermediate exists.

Sharding: 8 cores; core c handles batch b=c//4 and i-rows
[96*(c%4), 96*(c%4)+96).  Weights replicated.

Schedule (v2 — k-major unified stream):
 - All inputs stream as 10 chunk triggers chk[k] = [wi_k | wj_k | x_k]
   (229KB each, 1792B/partition descriptors), alternating the two HWDGE
   rings.  Chunk k's arrival enables ALL of layer-1 for contraction
   chunk k (psA t0/t1 + psB t0/t1, single-pass LDWEIGHTS), so layer 1
   is stream-paced and finishes ~0.5us after the last byte.
 - cst/bias rows ride the gpsimd SWDGE path (off the HW rings).
 - Biases enter the PSUM accumulations as rank-1 matmuls (ones ⊗ b).
 - Junk matmuls on a memset tile ramp the HAM clock before chunk 0.
 - at±-chain split: Vector owns t0 (fused PSUM reads), GpSimd owns t1
   (from SBUF relu copies), Scalar owns am1/bp relus.
 - Main contraction is j-block major: stationary = b±t j-block (M=128),
   moving = at± [n-half, i] (N=480).
 - bo + fp32->fp16 conversion fuse into one Vector broadcast-add per
   output block; blocks drain on alternating rings during the main loop.
"""

import numpy as np

import concourse.bass as bass
import concourse.mybir as mybir
import concourse.tile as tile
from concourse import bacc, bass_utils

# Problem constants (hardcoded per contract).
B, L, D, H, NB = 2, 384, 1280, 256, 10
P = 128
KT = D // P     # 10 contraction chunks of 128
HT = H // P     # 2 h-chunks of 128
NCORES = 8
IB = (B * L) // NCORES   # 96 i-rows per core
CW = 2 * H + L           # chunk width: wi(256) | wj(256) | x(384)

F32 = mybir.dt.float32
F16 = mybir.dt.float16
ALU = mybir.AluOpType
RELU = mybir.ActivationFunctionType.Relu

_last_result = None  # BassKernelResults of the most recent run (for test harness)


def build_nc():
    nc = bacc.Bacc("TRN2")

    # A-chunk pair g (k=2g,2g+1): per partition [wi_k(0:256) | xi_k(256:352)]
    cha = nc.dram_tensor("cha", [KT // 2, P, 2, 352], F16, kind="ExternalInput")
    # B-chunk pair g: per partition [wj_k(0:256) | x_k(256:640)]
    chb = nc.dram_tensor("chb", [KT // 2, P, 2, 640], F16, kind="ExternalInput")
    # cst[:, 0:2] = Wo per h-chunk, [:, 2:4] = -Wo, [:, 4] = bo replicated
    cst = nc.dram_tensor("cst", [P, 5, NB], F32, kind="ExternalInput")
    # bias rows on one partition: [bi_t0, bi_t1, bj_t0, bj_t1]
    brow = nc.dram_tensor("brow", [1, 4, P], F16, kind="ExternalInput")
    # [n-half, j-block, j, n, i]: output in j-major blocks (M=128 matmuls)
    out = nc.dram_tensor("out", [2, 3, P, NB // 2, IB], F16, kind="ExternalOutput")

    with tile.TileContext(nc) as tc:
        with (
            tc.tile_pool(name="persist", bufs=1) as pp,
            tc.tile_pool(name="psA", bufs=2, space="PSUM") as psA_pool,
            tc.tile_pool(name="psB", bufs=2, space="PSUM") as psB_pool,
            tc.tile_pool(name="psO", bufs=4, space="PSUM") as psO_pool,
            tc.tile_pool(name="stage", bufs=6) as stage_pool,
        ):
            tl = lambda shape, name, dt=F32: pp.tile(shape, dt, name=name, tag=name)
            cha_sb = tl([P, KT // 2, 2, 352], "cha_sb", F16)
            chb_sb = tl([P, KT // 2, 2, 640], "chb_sb", F16)
            cst_sb = tl([P, 5, NB], "cst_sb")
            brow_sb = tl([1, 4, P], "brow_sb", F16)
            ones_sb = tl([1, L], "ones_sb", F16)

            bp_sb = tl([P, HT, L], "bp_sb", F16)         # relu(xj+bj)      [h, j]
            bm_sb = tl([P, HT, L], "bm_sb", F16)         # relu(-(xj+bj))
            atp_sb = tl([P, HT, NB, IB], "atp_sb", F16)  # [h, n, i]
            atm_sb = tl([P, HT, NB, IB], "atm_sb", F16)
            ap1_sb = tl([P, IB], "ap1_sb")               # max(psA1, 0)
            am1_sb = tl([P, IB], "am1_sb")               # relu(-psA1)

            warm_sb = tl([P, L], "warm_sb", F16)
            # memsets on gpsimd: it is idle at program start, so the first
            # junk matmul (and the HAM clock ramp) starts ~1.5us earlier
            # than with vector memsets (vector waits on engine-main entry).
            nc.gpsimd.memset(warm_sb[:], 0.0)
            nc.gpsimd.memset(ones_sb[:], 1.0)

            def junk(n_junk):
                # Full 128-partition matmuls: the HAM clock monitor only
                # counts wide-K PE streaming (K=32/64 never ramps).
                psW = psO_pool.tile([IB, L], F32, name="psW", tag="psO")
                for _ in range(n_junk):
                    nc.tensor.matmul(psW[:], warm_sb[:, :IB], warm_sb[:],
                                     start=True, stop=True,
                                     skip_group_check=True)

            # ---- DMA triggers.  Emission order per engine = issue order.
            # Two-phase stream: A-chunks (wi + i-cols of x) first so psA
            # closes ~3us in and the long at±-chain overlaps the B-phase;
            # B-chunks (wj + full x) stream-pace psB, which closes right
            # before the main loop.  brow lands early on the scalar ring
            # (bias matmuls run mid-A-phase); cst mid-A on sync (needed
            # by the at±-chain at ~A-end).
            nc.sync.dma_start(cha_sb[:, 0], cha[0])
            nc.scalar.dma_start(cha_sb[:, 1], cha[1])
            nc.scalar.dma_start(brow_sb[:], brow[:])
            nc.sync.dma_start(cha_sb[:, 2], cha[2])
            nc.sync.dma_start(cst_sb[:], cst[:])
            nc.scalar.dma_start(cha_sb[:, 3], cha[3])
            nc.sync.dma_start(cha_sb[:, 4], cha[4])
            for g in range(KT // 2):
                eng = nc.scalar if g % 2 == 0 else nc.sync
                eng.dma_start(chb_sb[:, g], chb[g])

            psA = [psA_pool.tile([P, IB], F32, name="psA", tag="psA")
                   for _ in range(HT)]
            psB = [psB_pool.tile([P, L], F32, name="psB", tag="psB")
                   for _ in range(HT)]

            # ~3.5us of dense junk: the HAM un-throttle needs one fully-busy
            # 3.4us window of wide-N PE streaming, and the N=96 A-phase
            # matmuls never provide it.  This burns the dead stream-head
            # time and guarantees the A-phase runs at 2.4GHz.
            junk(11)

            # ---- layer 1 A-side: psA chunk-paced on the A-stream; bias
            # rank-1 joins after group 0; psA[0] closes first so the
            # Vector t0-chain starts while the t1 matmuls finish.
            def a_mm(g, j, t, start=False, stop=False):
                nc.tensor.matmul(psA[t][:], cha_sb[:, g, j, t * P:(t + 1) * P],
                                 cha_sb[:, g, j, H:H + IB],
                                 start=start, stop=stop)

            def b_mm(g, j, t, start=False, stop=False):
                nc.tensor.matmul(psB[t][:],
                                 chb_sb[:, g, j, t * P:(t + 1) * P],
                                 chb_sb[:, g, j, H:],
                                 start=start, stop=stop)

            for g in range(KT // 2):
                last = g == KT // 2 - 1
                if not last:
                    for j in range(2):
                        for t in range(HT):
                            a_mm(g, j, t, start=g == 0 and j == 0)
                else:
                    # close t0 before t1 so the t0 post-ops start first
                    a_mm(g, 0, 0)
                    a_mm(g, 0, 1)
                    a_mm(g, 1, 0, stop=True)
                    a_mm(g, 1, 1, stop=True)
                if g == 0:
                    for t in range(HT):
                        nc.tensor.matmul(psA[t][:], brow_sb[:, t],
                                         ones_sb[:, :IB],
                                         start=False, stop=False)
                junk(1)

            # ---- at±-chain: emitted here (= high scheduler priority) so
            # it runs concurrently with the B-phase stream/matmuls.
            wo_b = lambda s, lo, hi: cst_sb[:, s, lo:hi, None].to_broadcast(
                (P, hi - lo, IB))
            psa_b = lambda lo, hi: psA[0][:, None, :].to_broadcast(
                (P, hi - lo, IB))

            def at0_op(sign, lo, hi):
                # fused t0: atp = max(psA,0)*Wo ; atm = min(psA,0)*(-Wo)
                dst = (atp_sb if sign == 0 else atm_sb)[:, 0, lo:hi]
                op0 = ALU.max if sign == 0 else ALU.min
                nc.vector.scalar_tensor_tensor(dst, psa_b(lo, hi), 0.0,
                                               wo_b(2 * sign, lo, hi),
                                               op0, ALU.mult)

            def at1_op(sign, lo, hi):
                # t1 on gpsimd from SBUF; both a-parts non-negative -> +Wo.
                src = ap1_sb if sign == 0 else am1_sb
                dst = (atp_sb if sign == 0 else atm_sb)[:, 1, lo:hi]
                nc.gpsimd.tensor_tensor(
                    dst, src[:, None, :].to_broadcast((P, hi - lo, IB)),
                    wo_b(1, lo, hi), ALU.mult)

            nc.scalar.activation(am1_sb[:], psA[1][:], RELU, scale=-1.0)
            at0_op(0, 0, 5)
            nc.vector.tensor_scalar_max(ap1_sb[:], psA[1][:], 0.0)
            at0_op(1, 0, 5)
            at0_op(0, 5, 10)
            at0_op(1, 5, 10)
            at1_op(0, 0, 5)
            at1_op(1, 0, 5)
            at1_op(0, 5, 10)
            at1_op(1, 5, 10)

            # ---- layer 1 B-side: psB chunk-paced on the B-stream ----
            for g in range(KT // 2):
                last = g == KT // 2 - 1
                if not last:
                    for j in range(2):
                        for t in range(HT):
                            b_mm(g, j, t, start=g == 0 and j == 0)
                else:
                    b_mm(g, 0, 0)
                    b_mm(g, 0, 1)
                    b_mm(g, 1, 0, stop=True)
                    b_mm(g, 1, 1, stop=True)
                if g == 0:
                    for t in range(HT):
                        nc.tensor.matmul(psB[t][:], brow_sb[:, 2 + t],
                                         ones_sb[:],
                                         start=False, stop=False)
                junk(2)

            # ---- b± relus: scalar owns bp, vector owns bm ----
            nc.scalar.activation(bp_sb[:, 0], psB[0][:], RELU)
            nc.scalar.activation(bp_sb[:, 1], psB[1][:], RELU)
            nc.vector.tensor_scalar(bm_sb[:, 0], psB[0][:], -1.0, 0.0,
                                    ALU.mult, ALU.max)
            nc.vector.tensor_scalar(bm_sb[:, 1], psB[1][:], -1.0, 0.0,
                                    ALU.mult, ALU.max)

            # ---- main contraction, j-block major: stationary = b±t j-block
            # (M=128), moving = at± [n-half, i] (N=480).  Output bias
            # enters via cst row 4 during the fp32->fp16 PSUM drain.
            NH2 = NB // 2
            atp_v = atp_sb[:].rearrange("p t n i -> p t (n i)")
            atm_v = atm_sb[:].rearrange("p t n i -> p t (n i)")
            junk(1)
            for idx in range(6):
                nh, jb = idx // 3, idx % 3
                ns = slice(nh * NH2 * IB, (nh + 1) * NH2 * IB)
                js = slice(jb * P, (jb + 1) * P)
                psO = psO_pool.tile([P, NH2 * IB], F32, name="psO", tag="psO")
                nc.tensor.matmul(psO[:], bp_sb[:, 0, js], atp_v[:, 0, ns],
                                 start=True, stop=False)
                if idx == 0:
                    junk(2)  # bridge the at-chain latency, keep the clock up
                nc.tensor.matmul(psO[:], bm_sb[:, 0, js], atm_v[:, 0, ns],
                                 start=False, stop=False)
                if idx == 0:
                    junk(1)
                nc.tensor.matmul(psO[:], bp_sb[:, 1, js], atp_v[:, 1, ns],
                                 start=False, stop=False)
                if idx == 0:
                    junk(1)
                nc.tensor.matmul(psO[:], bm_sb[:, 1, js], atm_v[:, 1, ns],
                                 start=False, stop=True)
                # bias rides the fp16 convert as a broadcast add on Vector
                # (cst row 4 = bo replicated across partitions); no PE time.
                ostage = stage_pool.tile([P, NH2, IB], F16, name="ostage",
                                         tag="ostage")
                psO_3 = psO[:].rearrange("p (n i) -> p n i", i=IB)
                bo_b = lambda lo, hi: cst_sb[:, 4, nh * NH2 + lo:nh * NH2 + hi,
                                             None].to_broadcast((P, hi - lo, IB))
                eng = nc.scalar if idx % 2 == 0 else nc.sync
                if idx == 5:
                    # last block: convert/DMA in halves so the first half
                    # streams out while the second converts
                    nc.vector.tensor_tensor(ostage[:, 0:2], psO_3[:, 0:2],
                                            bo_b(0, 2), ALU.add)
                    eng.dma_start(out[nh, jb, :, 0:2], ostage[:, 0:2])
                    nc.vector.tensor_tensor(ostage[:, 2:NH2], psO_3[:, 2:NH2],
                                            bo_b(2, NH2), ALU.add)
                    nc.scalar.dma_start(out[nh, jb, :, 2:NH2], ostage[:, 2:NH2])
                else:
                    nc.vector.tensor_tensor(ostage[:], psO_3, bo_b(0, NH2),
                                            ALU.add)
                    eng.dma_start(out[nh, jb], ostage[:])

    return nc


def _prep_inputs(x, Wi, bi, Wj, bj, Wo, bo):
    """Build the 8 per-core input maps."""
    f = lambda a: np.ascontiguousarray(np.asarray(a, dtype=np.float32))
    x, Wi, bi, Wj, bj, Wo, bo = map(f, (x, Wi, bi, Wj, bj, Wo, bo))

    wi_k = Wi.astype(np.float16).reshape(KT, P, H)      # [k, 128, 256]
    wj_k = Wj.astype(np.float16).reshape(KT, P, H)

    wo_r = Wo.reshape(HT, P, NB).transpose(1, 0, 2)            # [128, 2, 10]
    cst = np.ascontiguousarray(np.stack(
        [wo_r[:, 0], wo_r[:, 1], -wo_r[:, 0], -wo_r[:, 1],
         np.tile(bo[None, :], (P, 1))], axis=1)).astype(np.float32)  # [128, 5, 10]
    brow = np.concatenate([bi.reshape(HT, P), bj.reshape(HT, P)],
                          axis=0)[None].astype(np.float16)     # [1, 4, 128]
    brow = np.ascontiguousarray(brow)

    xT = [x[b].T for b in range(B)]                            # [1280, 384]
    in_maps = []
    for c in range(NCORES):
        b, i0 = c // (NCORES // B), (c % (NCORES // B)) * IB
        xc = np.roll(xT[b], -i0, axis=1).astype(np.float16)    # i-cols first
        xk = xc.reshape(KT, P, L)                              # [k, 128, 384]
        # A-pairs: [wi_k | xi_k] ; B-pairs: [wj_k | x_k]
        cha = np.concatenate([wi_k, xk[:, :, :IB]], axis=2)    # [k, 128, 352]
        cha = np.ascontiguousarray(
            cha.reshape(KT // 2, 2, P, 352).transpose(0, 2, 1, 3))
        chb = np.concatenate([wj_k, xk], axis=2)               # [k, 128, 640]
        chb = np.ascontiguousarray(
            chb.reshape(KT // 2, 2, P, 640).transpose(0, 2, 1, 3))
        in_maps.append({"cha": cha, "chb": chb, "cst": cst, "brow": brow})
    return in_maps


def _run(inputs, trace=False):
    global _last_result
    nc = build_nc()
    if not nc.is_finalized():
        nc.finalize()
    in_maps = _prep_inputs(**inputs)
    res = bass_utils.run_bass_kernel_spmd(
        nc, in_maps, core_ids=list(range(NCORES)), trace=trace)
    _last_result = res
    full = np.empty((B, L, L, NB), dtype=np.float32)
    for c in range(NCORES):
        b, i0 = c // (NCORES // B), (c % (NCORES // B)) * IB
        o = res.results[c]["out"].astype(np.float32)   # [2, 3, 128, 5, 96]
        o = o.transpose(4, 1, 2, 0, 3).reshape(IB, L, NB)  # -> [i, j_rolled, n]
        full[b, i0:i0 + IB] = np.roll(o, i0, axis=1)
    return full


def kernel(**inputs):
    return _run(inputs, trace=False)
